# revision 34
# baseline (speedup 1.0000x reference)
"""ContraCLM token-level contrastive loss on 8 Trainium2 NeuronCores.

Data-parallel over the batch: core b handles sample b (B=8). Per core,
with S=1536, D=1024, T=0.05, the 2S x 2S exp-sim row sums are built
from three quadrant families, exploiting the symmetry of the full
matrix (only ~70% of the blocks are computed):

  A = f1 f1^T upper triangle: row r covers cols [128r, 1536); row sums
      go to view-1 rows directly, column sums of the strictly-upper
      part are accumulated (DVE, bf16) into cac1 and folded back to
      view-1 rows at the end (they stand in for the mirrored lower
      triangle).
  C = f2 f1^T full rows (computed instead of B so each row block only
      needs one fT2 tile -> no wait on the full view-2 transpose):
      row sums to view-2 rows, column sums into cac1 (these are the
      B-quadrant contributions to view-1 rows).  The diagonal is the
      positive-pair similarity: it is extracted into poss20 via a
      DVE multiply with identity + reduce, and KEPT in the row sum
      (denom = Ng + pos needs exactly that).
  D = f2 f2^T upper triangle, like A, col sums into cac2.

  Self-similarity diagonals get -1e9 added in PSUM before exp -> exact
  zero contribution.  Masked tokens have f=0 (mask folded into the
  rsqrt scale), so each masked column adds exp(0)=1: subtract
  K0 = 2S - 2n.  per_tok = log(rowsum + K0') - pos_sim/T; masked mean;
  each core returns its per-sample mean and the host averages the 8
  scalars (no device collective).

  fp8e4 (x8) DoubleRow matmuls, K=1024 in 4 double-k groups.  exp row
  sums ride the ScalarE activation free-dim accumulator.  View-2 norms
  (sum of squares) run on GpSimd+DVE, interleaved with A' so the
  scalar queue stays clear for exps.
"""

import sys

for _p in ("/opt/trn_rl_repo", "/opt/pypackages"):
    if _p not in sys.path:
        sys.path.append(_p)

from contextlib import ExitStack

import numpy as np

import bass_rust

import concourse.bass as bass
import concourse.tile as tile
from concourse import mybir
from concourse.bass_utils import run_bass_kernel_spmd
from concourse.masks import make_identity
from concourse.vector_clock import ScopedClock

# The walrus build in this container encodes at most 2 sync waits per
# instruction (bass_rust's inst_waits_full agrees), but Tile's semaphore
# assignment can attach more. Hoist excess waits onto unfusable same-engine
# NoOps immediately before the instruction — the engine executes its queue
# in order, so semantics are preserved.
_MAX_WAITS = 1


def _split_excess_waits(nc, ordered):
    for bb_name, insts in ordered.items():
        out = []
        changed = False
        for inst in insts:
            si = getattr(inst, "sync_info", None)
            waits = list(si.on_wait) if si is not None else []
            if len(waits) > _MAX_WAITS:
                changed = True
                extra, keep = waits[:-_MAX_WAITS], waits[-_MAX_WAITS:]
                for i in range(0, len(extra), _MAX_WAITS):
                    out.append(mybir.InstNoOp(
                        name=nc.get_next_instruction_name(),
                        sync_info=mybir.SyncInfo(
                            on_wait=extra[i:i + _MAX_WAITS], on_update=[]),
                        bass_nofuse=True,
                        engine=inst.engine,
                    ))
                si.on_wait = keep
            out.append(inst)
        if changed:
            insts[:] = out


_orig_lower_ordered_insts = tile.TileContext._lower_ordered_insts


def _patched_lower_ordered_insts(self, ordered):
    _split_excess_waits(self.nc, ordered)
    return _orig_lower_ordered_insts(self, ordered)


tile.TileContext._lower_ordered_insts = _patched_lower_ordered_insts


def _split_waits_drain_and_barrier(self, tick_clock, wait_clock):
    nc = self.nc
    probe = nc.sync.nop(nofuse=True)
    wait_clock.add_sem_waits(
        probe.ins, ScopedClock({None: tick_clock.global_clock}))
    si = probe.ins.sync_info
    waits = list(si.on_wait) if si is not None else []
    if len(waits) > _MAX_WAITS:
        si.on_wait = waits[:_MAX_WAITS]
        for i in range(_MAX_WAITS, len(waits), _MAX_WAITS):
            nxt = nc.sync.nop(nofuse=True)
            nxt.ins.sync_info = bass_rust.SyncInfo(
                on_wait=waits[i:i + _MAX_WAITS], on_update=[])
    nc.sync.drain()
    nc.all_engine_barrier()
    assert self.sems is not None
    popped = nc._tile_sem_poison_stack.pop()
    assert popped is self._sem_poison
    nc.clear_and_free_semaphores(list(self.sems.allocated().values()))
    nc.all_engine_barrier()


tile.TileContext._drain_and_barrier = _split_waits_drain_and_barrier

S, D, NCORES = 1536, 1024, 8
ST = S // 128            # 12 s-tiles per view
NB = 2 * ST              # 24 block rows of F
KT = D // 128            # 8 contraction tiles
TEMP_INV = 20.0          # 1 / 0.05
FP8_SCALE = 8.0          # f entries ~N(0, 1/32); x8 keeps them in e4m3's
                         # normal range (|f|*8 <~ 2, well under 240)
EXP_SCALE = TEMP_INV / (FP8_SCALE * FP8_SCALE)
F32 = mybir.dt.float32
BF16 = mybir.dt.bfloat16
FP8 = mybir.dt.float8e4
AF = mybir.ActivationFunctionType
ALU = mybir.AluOpType
DR = mybir.MatmulPerfMode.DoubleRow


def _build(num_devices: int = NCORES, debug_dump: bool = False) -> bass.Bass:
    nc = bass.Bass(num_devices=num_devices)
    h1 = nc.dram_tensor("h1", [S, D], F32, kind="ExternalInput")
    h2 = nc.dram_tensor("h2", [S, D], F32, kind="ExternalInput")
    # mask, pre-laid-out host-side as [128, ST] so token t = 128*col + row
    maskT = nc.dram_tensor("maskT", [128, ST], F32, kind="ExternalInput")
    out = nc.dram_tensor("loss", [1, 1], F32, kind="ExternalOutput")
    if debug_dump:
        ng_dump = nc.dram_tensor("ng_dump", [128, NB], F32,
                                 kind="ExternalOutput")
        poss_dump = nc.dram_tensor("poss_dump", [128, ST], F32,
                                   kind="ExternalOutput")
        sc8_dump = nc.dram_tensor("sc8_dump", [128, NB], F32,
                                  kind="ExternalOutput")

    with tile.TileContext(nc) as tc, ExitStack() as ctx:
        const_pool = ctx.enter_context(tc.tile_pool(name="const", bufs=1))
        big = ctx.enter_context(tc.tile_pool(name="big", bufs=1))
        stat = ctx.enter_context(tc.tile_pool(name="stat", bufs=1))

        h1k = big.tile([128, ST, D], F32)
        h2k = big.tile([128, ST, D], F32)
        fT1 = big.tile([128, KT, S], FP8)        # f1^T * 8, fp8e4
        fT2 = big.tile([128, KT, S], FP8)        # f2^T * 8

        msk = const_pool.tile([128, ST], F32)
        # input DMAs first: they are the long pole at startup
        nc.sync.dma_start(msk[:], maskT[:])
        for t in range(ST):
            nc.sync.dma_start(h1k[:, t, :], h1[t * 128:(t + 1) * 128, :])
        for t in range(ST):
            nc.sync.dma_start(h2k[:, t, :], h2[t * 128:(t + 1) * 128, :])

        identF = const_pool.tile([128, 128], F32)
        make_identity(nc, identF[:])
        identB = const_pool.tile([128, 128], BF16)
        make_identity(nc, identB[:])
        # -1e9 on the diagonal, bf16: injected into self-sim PSUM blocks
        # via an extra accumulating matmul (identB^T @ negIB = -1e9 I)
        negIB = const_pool.tile([128, 128], BF16)
        nc.gpsimd.memset(negIB[:], 0.0)
        nc.gpsimd.affine_select(
            out=negIB[:], in_=negIB[:], compare_op=ALU.not_equal,
            fill=-1e9, base=0, pattern=[[-1, 128]], channel_multiplier=1)
        ones_col = const_pool.tile([128, 1], F32)
        nc.gpsimd.memset(ones_col[:], 1.0)
        ones_sq = const_pool.tile([128, 128], F32)
        nc.gpsimd.memset(ones_sq[:], 1.0)
        ones_bf = const_pool.tile([128, 1], BF16)
        nc.gpsimd.memset(ones_bf[:], 1.0)

        ss = stat.tile([128, NB], F32)           # per-token sum of squares
        sc8 = stat.tile([128, NB], F32)          # 8 * mask * rsqrt(ss)
        nrm = stat.tile([128, NB], F32)
        acc = stat.tile([128, NB, 2], F32)       # per-strip row sums
        cac1 = stat.tile([128, S], BF16)         # col acc -> view-1 rows
        cac2 = stat.tile([128, S], BF16)         # col acc -> view-2 rows
        poss20 = stat.tile([128, ST], F32)       # 64 * pos_sim
        msk24 = stat.tile([128, NB], F32)
        negK0 = stat.tile([128, 1], F32)
        recn = stat.tile([1, 1], F32)

        nc.gpsimd.memset(acc[:], 0.0)
        nc.gpsimd.memset(cac1[:], 0.0)
        nc.gpsimd.memset(cac2[:], 0.0)

        # ---- mask-only precomputes ----
        with tc.tile_pool(name="ep0", bufs=1) as ep0, \
             tc.tile_pool(name="ep0_ps", bufs=1, space="PSUM") as ep0p:
            msum = ep0.tile([128, 1], F32)
            nc.vector.tensor_reduce(msum[:], msk[:],
                                    axis=mybir.AxisListType.X, op=ALU.add)
            nps = ep0p.tile([128, 1], F32)
            nc.tensor.matmul(nps[:], ones_sq[:], msum[:], start=True,
                             stop=True)
            # -K0 = 2n - 2S
            nc.scalar.activation(negK0[:], nps[:], AF.Copy, scale=2.0,
                                 bias=float(-2 * S))
            n2c = ep0.tile([1, 1], F32)
            nc.scalar.activation(n2c[:], nps[0:1, :], AF.Copy, scale=2.0)
            nc.vector.reciprocal(recn[:], n2c[:])   # 1 / (2n)
            nc.vector.tensor_copy(msk24[:, 0:ST], msk[:])
            nc.vector.tensor_copy(msk24[:, ST:NB], msk[:])

        def finish_scale(o, n):
            """sc8[:, o:o+n] = 8 * msk * rsqrt(ss[:, o:o+n])."""
            nc.scalar.activation(nrm[:, o:o + n], ss[:, o:o + n], AF.Sqrt)
            ri = stat.tile([128, n], F32, name=f"ri_{o}")
            nc.vector.reciprocal(ri[:], nrm[:, o:o + n])
            rm = stat.tile([128, n], F32, name=f"rm_{o}")
            nc.vector.tensor_mul(rm[:], ri[:], msk24[:, o:o + n])
            nc.vector.tensor_scalar_mul(sc8[:, o:o + n], rm[:], FP8_SCALE)

        def transpose_tile(hk, fT, half, t, tps, scr, kg1_scalar,
                           kg0_scalar=False, fn_gpsimd=False):
            """fT[:, :, t*128:+128] = (hk[:,t,:] * sc8)^T as fp8."""
            o = half * ST
            fn = scr.tile([128, D], BF16, tag="fn", name=f"fn_{half}_{t}")
            nc.scalar.activation(fn[:, 0:D // 2], hk[:, t, 0:D // 2],
                                 AF.Copy, scale=sc8[:, o + t:o + t + 1])
            eng = nc.gpsimd if fn_gpsimd else nc.vector
            eng.tensor_scalar_mul(fn[:, D // 2:D], hk[:, t, D // 2:D],
                                  sc8[:, o + t:o + t + 1])
            c0 = t * 128
            for kg in range(2):
                pt = tps.tile([128, 512], BF16, tag="pt",
                              name=f"pt_{half}_{t}_{kg}")
                for j in range(4):
                    k = kg * 4 + j
                    nc.tensor.transpose(pt[:, j * 128:(j + 1) * 128],
                                        fn[:, k * 128:(k + 1) * 128],
                                        identB[:])
                dst = fT[:, kg * 4:(kg + 1) * 4, c0:c0 + 128]
                src = pt[:].rearrange("p (j c) -> p j c", j=4)
                use_scalar = (kg1_scalar if kg == 1 else kg0_scalar)
                if use_scalar:
                    nc.scalar.copy(dst, src)
                else:
                    nc.vector.tensor_copy(dst, src)

        def mm_strip(ps, lhsT, rT, rhsT, col0, ncols):
            """sim strip into ps[:, 0:ncols] (DoubleRow, K=1024)."""
            for g in range(KT // 2):
                u0 = 0
                while u0 < ncols:
                    u1 = min(u0 + 512, ncols)
                    nc.tensor.matmul(
                        ps[:, u0:u1],
                        lhsT[:, 2 * g:2 * g + 2, rT * 128:(rT + 1) * 128],
                        rhsT[:, 2 * g:2 * g + 2, col0 + u0:col0 + u1],
                        perf_mode=DR,
                        start=(g == 0), stop=(g == KT // 2 - 1))
                    u0 = u1

        # ---- phase A: view-1 norms (scalar) + transpose, in halves;
        # view-2 squares (GpSimd) ride along as h2 tiles land ----
        with tc.tile_pool(name="sqpA", bufs=2) as sqp, \
             tc.tile_pool(name="scrA", bufs=3) as scr, \
             tc.tile_pool(name="tpA_ps", bufs=2, space="PSUM") as tps:
            wr = tps.tile([128, 128], BF16, tag="warm", name="warm")

            def pe_keepalive(n):
                # dependency-free transposes: execute only when the
                # tensor queue would otherwise idle, keeping the HAM
                # clock gate at 2.4GHz through the DVE-paced stretches
                for _ in range(n):
                    nc.tensor.transpose(wr[:], identB[:], identB[:])

            pe_keepalive(80)
            for hf in range(2):
                t0 = hf * (ST // 2)
                for t in range(t0, t0 + ST // 2):
                    sq = sqp.tile([128, D], BF16, tag="sq", name=f"sqA_{t}")
                    nc.scalar.activation(sq[:], h1k[:, t, :], AF.Square,
                                         accum_out=ss[:, t:t + 1])
                finish_scale(t0, ST // 2)
                for t in range(t0, t0 + ST // 2):
                    transpose_tile(h1k, fT1, 0, t, tps, scr,
                                   kg1_scalar=False, fn_gpsimd=True)
                    pe_keepalive(20)

        # view-2 norms (scalar Square+accum) before A' exps hit the
        # scalar queue, so sc8_2 is ready early for the B transposes
        with tc.tile_pool(name="sqpB", bufs=2) as sqpB:
            for t in range(ST):
                sq = sqpB.tile([128, D], BF16, tag="sq", name=f"sqB_{t}")
                nc.scalar.activation(sq[:], h2k[:, t, :], AF.Square,
                                     accum_out=ss[:, ST + t:ST + t + 1])
        finish_scale(ST, ST)

        # ---- A' (A-quadrant upper triangle), phase-B transposes and
        # C rows interleaved to keep TensorE continuously busy ----
        with ExitStack() as bctx:
            psA = bctx.enter_context(
                tc.tile_pool(name="psA", bufs=2, space="PSUM"))
            esA = bctx.enter_context(tc.tile_pool(name="esA", bufs=3))
            scrB = bctx.enter_context(tc.tile_pool(name="scrB", bufs=3))
            dvB = bctx.enter_context(tc.tile_pool(name="dvB", bufs=2))
            if True:

                def a_row(r):
                    ncols = S - r * 128
                    trip = psA.tile([128, S], F32, tag="tp",
                                    name=f"tpA_{r}")
                    mm_strip(trip, fT1, r, fT1, r * 128, ncols)
                    nc.tensor.matmul(trip[:, 0:128], identB[:], negIB[:],
                                     start=False, stop=True,
                                     skip_group_check=True)
                    es = esA.tile([128, S], BF16, tag="es",
                                  name=f"esA_{r}")
                    nc.scalar.activation(es[:, 0:ncols], trip[:, 0:ncols],
                                         AF.Exp, scale=EXP_SCALE,
                                         accum_out=acc[:, r, 0:1])
                    if ncols > 128:
                        nc.gpsimd.tensor_add(cac1[:, (r + 1) * 128:S],
                                             cac1[:, (r + 1) * 128:S],
                                             es[:, 128:ncols])

                def c_row(rT):
                    trip = psA.tile([128, S], F32, tag="tp",
                                    name=f"tpC_{rT}")
                    mm_strip(trip, fT2, rT, fT1, 0, S)
                    # counterpart diagonal: extract 64*pos_sim, keep it
                    # inside the row sum (denom = Ng + pos)
                    dscr = dvB.tile([128, 128], F32, tag="dg",
                                    name=f"dg_{rT}")
                    nc.vector.tensor_mul(
                        dscr[:], trip[:, rT * 128:(rT + 1) * 128],
                        identF[:])
                    nc.vector.tensor_reduce(
                        poss20[:, rT:rT + 1], dscr[:],
                        axis=mybir.AxisListType.X, op=ALU.add)
                    es = esA.tile([128, S], BF16, tag="es",
                                  name=f"esC_{rT}")
                    nc.scalar.activation(es[:], trip[:], AF.Exp,
                                         scale=EXP_SCALE,
                                         accum_out=acc[:, ST + rT, 0:1])
                    nc.vector.tensor_add(cac1[:], cac1[:], es[:])

                def d_row(rT):
                    ncols = S - rT * 128
                    trip = psA.tile([128, S], F32, tag="tp",
                                    name=f"tpD_{rT}")
                    mm_strip(trip, fT2, rT, fT2, rT * 128, ncols)
                    nc.tensor.matmul(trip[:, 0:128], identB[:], negIB[:],
                                     start=False, stop=True,
                                     skip_group_check=True)
                    es = esA.tile([128, S], BF16, tag="es",
                                  name=f"esD_{rT}")
                    nc.scalar.activation(es[:, 0:ncols], trip[:, 0:ncols],
                                         AF.Exp, scale=EXP_SCALE,
                                         accum_out=acc[:, ST + rT, 1:2])
                    if ncols > 128:
                        nc.vector.tensor_add(cac2[:, (rT + 1) * 128:S],
                                             cac2[:, (rT + 1) * 128:S],
                                             es[:, 128:ncols])

                def fold(cac, half):
                    for jb in range(ST):
                        nc.tensor.matmul(
                            pcbt[:, half, jb:jb + 1],
                            cac[:, jb * 128:(jb + 1) * 128],
                            ones_bf[:], start=True, stop=True,
                            skip_group_check=True)

                with tc.tile_pool(name="tpB_ps", bufs=2,
                                  space="PSUM") as tpsB:
                    for r in range(6):
                        a_row(r)
                    for r in range(6, ST):
                        a_row(r)
                        transpose_tile(h2k, fT2, 1, r - 6, tpsB, scrB,
                                       kg1_scalar=False)
                    for i in range(6):
                        transpose_tile(h2k, fT2, 1, 6 + i, tpsB, scrB,
                                       kg1_scalar=False)
                        c_row(i)
                cbp = bctx.enter_context(
                    tc.tile_pool(name="cb_ps", bufs=1, space="PSUM"))
                pcbt = cbp.tile([128, 2, ST], F32, name="pcbt")
                for rT in range(6, ST):
                    c_row(rT)
                d_row(0)
                d_row(1)
                d_row(2)
                fold(cac1, 0)
                for rT in range(3, ST):
                    d_row(rT)
                fold(cac2, 1)

            # ---- epilogue ----
            with tc.tile_pool(name="ep", bufs=1) as ep, \
                 tc.tile_pool(name="ep_ps", bufs=1, space="PSUM") as epp:
                ng = ep.tile([128, NB], F32)
                nc.vector.tensor_reduce(ng[:], acc[:],
                                        axis=mybir.AxisListType.X,
                                        op=ALU.add)
                nc.vector.tensor_add(ng[:, 0:ST], ng[:, 0:ST],
                                     pcbt[:, 0, :])
                nc.vector.tensor_add(ng[:, ST:NB], ng[:, ST:NB],
                                     pcbt[:, 1, :])
                denom = ep.tile([128, NB], F32)
                nc.vector.tensor_scalar_add(denom[:], ng[:], negK0[:])
                lg = ep.tile([128, NB], F32)
                nc.scalar.activation(lg[:], denom[:], AF.Ln)
                if debug_dump:
                    nc.sync.dma_start(ng_dump[:], ng[:])
                    nc.sync.dma_start(poss_dump[:], poss20[:])
                    nc.sync.dma_start(sc8_dump[:], sc8[:])
                ptok = ep.tile([128, NB], F32)
                nc.vector.tensor_mul(ptok[:], lg[:], msk24[:])
                p20m = ep.tile([128, ST], F32)
                nc.vector.tensor_mul(p20m[:], poss20[:], msk[:])
                # poss20 held 64*pos_sim (raw psum); scale to pos_sim/T
                nc.vector.tensor_scalar_mul(p20m[:], p20m[:], EXP_SCALE)
                nc.vector.tensor_sub(ptok[:, 0:ST], ptok[:, 0:ST],
                                     p20m[:])
                nc.vector.tensor_sub(ptok[:, ST:NB], ptok[:, ST:NB],
                                     p20m[:])
                tsum = ep.tile([128, 1], F32)
                nc.vector.tensor_reduce(tsum[:], ptok[:],
                                        axis=mybir.AxisListType.X,
                                        op=ALU.add)
                lps = epp.tile([1, 1], F32)
                nc.tensor.matmul(lps[:], ones_col[:], tsum[:], start=True,
                                 stop=True)
                lsb = ep.tile([1, 1], F32)
                nc.vector.tensor_mul(lsb[:], lps[:], recn[:])
                nc.sync.dma_start(out[:], lsb[:])

    return nc


_NC = None


def _mask_layout(mask_row: np.ndarray) -> np.ndarray:
    # token t = 128 * col + row  ->  [128, ST]
    return np.ascontiguousarray(
        mask_row.astype(np.float32).reshape(ST, 128).T)


def kernel(last_hidden_states_1, last_hidden_states_2, token_mask_batch):
    global _NC
    h1 = np.ascontiguousarray(np.asarray(last_hidden_states_1,
                                         dtype=np.float32))
    h2 = np.ascontiguousarray(np.asarray(last_hidden_states_2,
                                         dtype=np.float32))
    mask = np.asarray(token_mask_batch)
    assert h1.shape == (NCORES, S, D), h1.shape

    if _NC is None:
        _NC = _build(NCORES)

    in_maps = [
        {"h1": h1[b], "h2": h2[b], "maskT": _mask_layout(mask[b])}
        for b in range(NCORES)
    ]
    res = run_bass_kernel_spmd(_NC, in_maps, list(range(NCORES)))
    losses = [float(np.asarray(res.results[b]["loss"]).reshape(()))
              for b in range(NCORES)]
    return np.float32(np.mean(losses))


# revision 35
# speedup vs baseline: 1.2342x; 1.2342x over previous
"""ContraCLM token-level contrastive loss on 8 Trainium2 NeuronCores.

Data-parallel over the batch: core b handles sample b (B=8). Per core,
with S=1536, D=1024, T=0.05, the 2S x 2S exp-sim row sums are built
from three quadrant families, exploiting the symmetry of the full
matrix (only ~70% of the blocks are computed):

  A = f1 f1^T upper triangle: row r covers cols [128r, 1536); row sums
      go to view-1 rows directly, column sums of the strictly-upper
      part are accumulated (DVE, bf16) into cac1 and folded back to
      view-1 rows at the end (they stand in for the mirrored lower
      triangle).
  C = f2 f1^T full rows (computed instead of B so each row block only
      needs one fT2 tile -> no wait on the full view-2 transpose):
      row sums to view-2 rows, column sums into cac1 (these are the
      B-quadrant contributions to view-1 rows).  The diagonal is the
      positive-pair similarity: it is extracted into poss20 via a
      DVE multiply with identity + reduce, and KEPT in the row sum
      (denom = Ng + pos needs exactly that).
  D = f2 f2^T upper triangle, like A, col sums into cac2.

  Self-similarity diagonals get -1e9 added in PSUM before exp -> exact
  zero contribution.  Masked tokens have f=0 (mask folded into the
  rsqrt scale), so each masked column adds exp(0)=1: subtract
  K0 = 2S - 2n.  per_tok = log(rowsum + K0') - pos_sim/T; masked mean;
  each core returns its per-sample mean and the host averages the 8
  scalars (no device collective).

  fp8e4 (x8) DoubleRow matmuls, K=1024 in 4 double-k groups.  exp row
  sums ride the ScalarE activation free-dim accumulator.  View-2 norms
  (sum of squares) run on GpSimd+DVE, interleaved with A' so the
  scalar queue stays clear for exps.
"""

import sys

for _p in ("/opt/trn_rl_repo", "/opt/pypackages"):
    if _p not in sys.path:
        sys.path.append(_p)

from contextlib import ExitStack

import numpy as np

import bass_rust

import concourse.bass as bass
import concourse.tile as tile
from concourse import mybir
from concourse.bass_utils import run_bass_kernel_spmd
from concourse.masks import make_identity
from concourse.vector_clock import ScopedClock

# The walrus build in this container encodes at most 2 sync waits per
# instruction (bass_rust's inst_waits_full agrees), but Tile's semaphore
# assignment can attach more. Hoist excess waits onto unfusable same-engine
# NoOps immediately before the instruction — the engine executes its queue
# in order, so semantics are preserved.
_MAX_WAITS = 1


def _split_excess_waits(nc, ordered):
    for bb_name, insts in ordered.items():
        out = []
        changed = False
        for inst in insts:
            si = getattr(inst, "sync_info", None)
            waits = list(si.on_wait) if si is not None else []
            if len(waits) > _MAX_WAITS:
                changed = True
                extra, keep = waits[:-_MAX_WAITS], waits[-_MAX_WAITS:]
                for i in range(0, len(extra), _MAX_WAITS):
                    out.append(mybir.InstNoOp(
                        name=nc.get_next_instruction_name(),
                        sync_info=mybir.SyncInfo(
                            on_wait=extra[i:i + _MAX_WAITS], on_update=[]),
                        bass_nofuse=True,
                        engine=inst.engine,
                    ))
                si.on_wait = keep
            out.append(inst)
        if changed:
            insts[:] = out


_orig_lower_ordered_insts = tile.TileContext._lower_ordered_insts


def _patched_lower_ordered_insts(self, ordered):
    _split_excess_waits(self.nc, ordered)
    return _orig_lower_ordered_insts(self, ordered)


tile.TileContext._lower_ordered_insts = _patched_lower_ordered_insts


def _split_waits_drain_and_barrier(self, tick_clock, wait_clock):
    nc = self.nc
    probe = nc.sync.nop(nofuse=True)
    wait_clock.add_sem_waits(
        probe.ins, ScopedClock({None: tick_clock.global_clock}))
    si = probe.ins.sync_info
    waits = list(si.on_wait) if si is not None else []
    if len(waits) > _MAX_WAITS:
        si.on_wait = waits[:_MAX_WAITS]
        for i in range(_MAX_WAITS, len(waits), _MAX_WAITS):
            nxt = nc.sync.nop(nofuse=True)
            nxt.ins.sync_info = bass_rust.SyncInfo(
                on_wait=waits[i:i + _MAX_WAITS], on_update=[])
    nc.sync.drain()
    nc.all_engine_barrier()
    assert self.sems is not None
    popped = nc._tile_sem_poison_stack.pop()
    assert popped is self._sem_poison
    nc.clear_and_free_semaphores(list(self.sems.allocated().values()))
    nc.all_engine_barrier()


tile.TileContext._drain_and_barrier = _split_waits_drain_and_barrier

S, D, NCORES = 1536, 1024, 8
ST = S // 128            # 12 s-tiles per view
NB = 2 * ST              # 24 block rows of F
KT = D // 128            # 8 contraction tiles
TEMP_INV = 20.0          # 1 / 0.05
FP8_SCALE = 8.0          # f entries ~N(0, 1/32); x8 keeps them in e4m3's
                         # normal range (|f|*8 <~ 2, well under 240)
EXP_SCALE = TEMP_INV / (FP8_SCALE * FP8_SCALE)
F32 = mybir.dt.float32
BF16 = mybir.dt.bfloat16
FP8 = mybir.dt.float8e4
AF = mybir.ActivationFunctionType
ALU = mybir.AluOpType
DR = mybir.MatmulPerfMode.DoubleRow


def _build(num_devices: int = NCORES, debug_dump: bool = False) -> bass.Bass:
    nc = bass.Bass(num_devices=num_devices)
    h1 = nc.dram_tensor("h1", [S, D], F32, kind="ExternalInput")
    h2 = nc.dram_tensor("h2", [S, D], F32, kind="ExternalInput")
    # mask, pre-laid-out host-side as [128, ST] so token t = 128*col + row
    maskT = nc.dram_tensor("maskT", [128, ST], F32, kind="ExternalInput")
    out = nc.dram_tensor("loss", [1, 1], F32, kind="ExternalOutput")
    if debug_dump:
        ng_dump = nc.dram_tensor("ng_dump", [128, NB], F32,
                                 kind="ExternalOutput")
        poss_dump = nc.dram_tensor("poss_dump", [128, ST], F32,
                                   kind="ExternalOutput")
        sc8_dump = nc.dram_tensor("sc8_dump", [128, NB], F32,
                                  kind="ExternalOutput")

    with tile.TileContext(nc) as tc, ExitStack() as ctx:
        const_pool = ctx.enter_context(tc.tile_pool(name="const", bufs=1))
        big = ctx.enter_context(tc.tile_pool(name="big", bufs=1))
        stat = ctx.enter_context(tc.tile_pool(name="stat", bufs=1))

        h1k = big.tile([128, ST, D], F32)
        h2k = big.tile([128, ST, D], F32)
        fT1 = big.tile([128, KT, S], FP8)        # f1^T * 8, fp8e4
        fT2 = big.tile([128, KT, S], FP8)        # f2^T * 8

        msk = const_pool.tile([128, ST], F32)
        # input DMAs first: they are the long pole at startup
        nc.sync.dma_start(msk[:], maskT[:])
        for t in range(ST):
            nc.sync.dma_start(h1k[:, t, :], h1[t * 128:(t + 1) * 128, :])
        for t in range(ST):
            nc.sync.dma_start(h2k[:, t, :], h2[t * 128:(t + 1) * 128, :])

        identF = const_pool.tile([128, 128], F32)
        make_identity(nc, identF[:])
        identB = const_pool.tile([128, 128], BF16)
        make_identity(nc, identB[:])
        # -1e9 on the diagonal, bf16: injected into self-sim PSUM blocks
        # via an extra accumulating matmul (identB^T @ negIB = -1e9 I)
        negIB = const_pool.tile([128, 128], BF16)
        nc.gpsimd.memset(negIB[:], 0.0)
        nc.gpsimd.affine_select(
            out=negIB[:], in_=negIB[:], compare_op=ALU.not_equal,
            fill=-1e9, base=0, pattern=[[-1, 128]], channel_multiplier=1)
        ones_col = const_pool.tile([128, 1], F32)
        nc.gpsimd.memset(ones_col[:], 1.0)
        ones_sq = const_pool.tile([128, 128], F32)
        nc.gpsimd.memset(ones_sq[:], 1.0)
        ones_bf = const_pool.tile([128, 1], BF16)
        nc.gpsimd.memset(ones_bf[:], 1.0)

        ss = stat.tile([128, NB], F32)           # per-token sum of squares
        sc8 = stat.tile([128, NB], F32)          # 8 * mask * rsqrt(ss)
        nrm = stat.tile([128, NB], F32)
        acc = stat.tile([128, NB, 2], F32)       # per-strip row sums
        cac1 = stat.tile([128, S], BF16)         # col acc -> view-1 rows
        cac2 = stat.tile([128, S], BF16)         # col acc -> view-2 rows
        poss20 = stat.tile([128, ST], F32)       # 64 * pos_sim
        msk24 = stat.tile([128, NB], F32)
        negK0 = stat.tile([128, 1], F32)
        recn = stat.tile([1, 1], F32)

        nc.gpsimd.memset(acc[:], 0.0)
        nc.gpsimd.memset(cac1[:], 0.0)
        nc.gpsimd.memset(cac2[:], 0.0)

        # ---- mask-only precomputes ----
        with tc.tile_pool(name="ep0", bufs=1) as ep0, \
             tc.tile_pool(name="ep0_ps", bufs=1, space="PSUM") as ep0p:
            msum = ep0.tile([128, 1], F32)
            nc.vector.tensor_reduce(msum[:], msk[:],
                                    axis=mybir.AxisListType.X, op=ALU.add)
            nps = ep0p.tile([128, 1], F32)
            nc.tensor.matmul(nps[:], ones_sq[:], msum[:], start=True,
                             stop=True)
            # -K0 = 2n - 2S
            nc.scalar.activation(negK0[:], nps[:], AF.Copy, scale=2.0,
                                 bias=float(-2 * S))
            n2c = ep0.tile([1, 1], F32)
            nc.scalar.activation(n2c[:], nps[0:1, :], AF.Copy, scale=2.0)
            nc.vector.reciprocal(recn[:], n2c[:])   # 1 / (2n)
            nc.vector.tensor_copy(msk24[:, 0:ST], msk[:])
            nc.vector.tensor_copy(msk24[:, ST:NB], msk[:])

        def finish_scale(o, n):
            """sc8[:, o:o+n] = 8 * msk * rsqrt(ss[:, o:o+n])."""
            nc.scalar.activation(nrm[:, o:o + n], ss[:, o:o + n], AF.Sqrt)
            ri = stat.tile([128, n], F32, name=f"ri_{o}")
            nc.vector.reciprocal(ri[:], nrm[:, o:o + n])
            rm = stat.tile([128, n], F32, name=f"rm_{o}")
            nc.vector.tensor_mul(rm[:], ri[:], msk24[:, o:o + n])
            nc.vector.tensor_scalar_mul(sc8[:, o:o + n], rm[:], FP8_SCALE)

        def transpose_tile(hk, fT, half, t, tps, scr, kg1_scalar,
                           kg0_scalar=False):
            """fT[:, :, t*128:+128] = (hk[:,t,:] * sc8)^T as fp8."""
            o = half * ST
            fn = scr.tile([128, D], BF16, tag="fn", name=f"fn_{half}_{t}")
            nc.scalar.activation(fn[:, 0:D // 2], hk[:, t, 0:D // 2],
                                 AF.Copy, scale=sc8[:, o + t:o + t + 1])
            nc.vector.tensor_scalar_mul(fn[:, D // 2:D],
                                        hk[:, t, D // 2:D],
                                        sc8[:, o + t:o + t + 1])
            c0 = t * 128
            for kg in range(2):
                pt = tps.tile([128, 512], BF16, tag="pt",
                              name=f"pt_{half}_{t}_{kg}")
                for j in range(4):
                    k = kg * 4 + j
                    nc.tensor.transpose(pt[:, j * 128:(j + 1) * 128],
                                        fn[:, k * 128:(k + 1) * 128],
                                        identB[:])
                dst = fT[:, kg * 4:(kg + 1) * 4, c0:c0 + 128]
                src = pt[:].rearrange("p (j c) -> p j c", j=4)
                use_scalar = (kg1_scalar if kg == 1 else kg0_scalar)
                if use_scalar:
                    nc.scalar.copy(dst, src)
                else:
                    nc.vector.tensor_copy(dst, src)

        def mm_strip(ps, lhsT, rT, rhsT, col0, ncols):
            """sim strip into ps[:, 0:ncols] (DoubleRow, K=1024)."""
            for g in range(KT // 2):
                u0 = 0
                while u0 < ncols:
                    u1 = min(u0 + 512, ncols)
                    nc.tensor.matmul(
                        ps[:, u0:u1],
                        lhsT[:, 2 * g:2 * g + 2, rT * 128:(rT + 1) * 128],
                        rhsT[:, 2 * g:2 * g + 2, col0 + u0:col0 + u1],
                        perf_mode=DR,
                        start=(g == 0), stop=(g == KT // 2 - 1))
                    u0 = u1

        # ---- phase A: view-1 norms (scalar) + transpose, in halves;
        # view-2 squares (GpSimd) ride along as h2 tiles land ----
        with tc.tile_pool(name="sqpA", bufs=2) as sqp, \
             tc.tile_pool(name="scrA", bufs=3) as scr, \
             tc.tile_pool(name="tpA_ps", bufs=2, space="PSUM") as tps:
            wr = tps.tile([128, 128], BF16, tag="warm", name="warm")

            def pe_keepalive(n):
                # dependency-free transposes: execute only when the
                # tensor queue would otherwise idle, keeping the HAM
                # clock gate at 2.4GHz through the DVE-paced stretches
                for _ in range(n):
                    nc.tensor.transpose(wr[:], identB[:], identB[:])

            pe_keepalive(80)
            for hf in range(2):
                t0 = hf * (ST // 2)
                for t in range(t0, t0 + ST // 2):
                    sq = sqp.tile([128, D], BF16, tag="sq", name=f"sqA_{t}")
                    nc.scalar.activation(sq[:], h1k[:, t, :], AF.Square,
                                         accum_out=ss[:, t:t + 1])
                finish_scale(t0, ST // 2)
                for t in range(t0, t0 + ST // 2):
                    transpose_tile(h1k, fT1, 0, t, tps, scr,
                                   kg1_scalar=False)
                    pe_keepalive(20)

        # view-2 norms (scalar Square+accum) before A' exps hit the
        # scalar queue, so sc8_2 is ready early for the B transposes
        with tc.tile_pool(name="sqpB", bufs=2) as sqpB:
            for t in range(ST):
                sq = sqpB.tile([128, D], BF16, tag="sq", name=f"sqB_{t}")
                nc.scalar.activation(sq[:], h2k[:, t, :], AF.Square,
                                     accum_out=ss[:, ST + t:ST + t + 1])
        finish_scale(ST, ST)

        # ---- A' (A-quadrant upper triangle), phase-B transposes and
        # C rows interleaved to keep TensorE continuously busy ----
        with ExitStack() as bctx:
            psA = bctx.enter_context(
                tc.tile_pool(name="psA", bufs=2, space="PSUM"))
            esA = bctx.enter_context(tc.tile_pool(name="esA", bufs=3))
            scrB = bctx.enter_context(tc.tile_pool(name="scrB", bufs=3))
            dvB = bctx.enter_context(tc.tile_pool(name="dvB", bufs=2))
            if True:

                def a_row(r):
                    ncols = S - r * 128
                    trip = psA.tile([128, S], F32, tag="tp",
                                    name=f"tpA_{r}")
                    mm_strip(trip, fT1, r, fT1, r * 128, ncols)
                    nc.tensor.matmul(trip[:, 0:128], identB[:], negIB[:],
                                     start=False, stop=True,
                                     skip_group_check=True)
                    es = esA.tile([128, S], BF16, tag="es",
                                  name=f"esA_{r}")
                    nc.scalar.activation(es[:, 0:ncols], trip[:, 0:ncols],
                                         AF.Exp, scale=EXP_SCALE,
                                         accum_out=acc[:, r, 0:1])
                    if ncols > 128:
                        nc.gpsimd.tensor_add(cac1[:, (r + 1) * 128:S],
                                             cac1[:, (r + 1) * 128:S],
                                             es[:, 128:ncols])

                def c_row(rT):
                    trip = psA.tile([128, S], F32, tag="tp",
                                    name=f"tpC_{rT}")
                    mm_strip(trip, fT2, rT, fT1, 0, S)
                    # counterpart diagonal: extract 64*pos_sim, keep it
                    # inside the row sum (denom = Ng + pos)
                    dscr = dvB.tile([128, 128], F32, tag="dg",
                                    name=f"dg_{rT}")
                    nc.vector.tensor_mul(
                        dscr[:], trip[:, rT * 128:(rT + 1) * 128],
                        identF[:])
                    nc.vector.tensor_reduce(
                        poss20[:, rT:rT + 1], dscr[:],
                        axis=mybir.AxisListType.X, op=ALU.add)
                    es = esA.tile([128, S], BF16, tag="es",
                                  name=f"esC_{rT}")
                    nc.scalar.activation(es[:], trip[:], AF.Exp,
                                         scale=EXP_SCALE,
                                         accum_out=acc[:, ST + rT, 0:1])
                    nc.vector.tensor_add(cac1[:], cac1[:], es[:])

                def d_row(rT):
                    ncols = S - rT * 128
                    trip = psA.tile([128, S], F32, tag="tp",
                                    name=f"tpD_{rT}")
                    mm_strip(trip, fT2, rT, fT2, rT * 128, ncols)
                    nc.tensor.matmul(trip[:, 0:128], identB[:], negIB[:],
                                     start=False, stop=True,
                                     skip_group_check=True)
                    es = esA.tile([128, S], BF16, tag="es",
                                  name=f"esD_{rT}")
                    nc.scalar.activation(es[:, 0:ncols], trip[:, 0:ncols],
                                         AF.Exp, scale=EXP_SCALE,
                                         accum_out=acc[:, ST + rT, 1:2])
                    if ncols > 128:
                        nc.vector.tensor_add(cac2[:, (rT + 1) * 128:S],
                                             cac2[:, (rT + 1) * 128:S],
                                             es[:, 128:ncols])

                def fold(cac, half):
                    for jb in range(ST):
                        nc.tensor.matmul(
                            pcbt[:, half, jb:jb + 1],
                            cac[:, jb * 128:(jb + 1) * 128],
                            ones_bf[:], start=True, stop=True,
                            skip_group_check=True)

                with tc.tile_pool(name="tpB_ps", bufs=2,
                                  space="PSUM") as tpsB:
                    for r in range(6):
                        a_row(r)
                    for r in range(6, ST):
                        a_row(r)
                        transpose_tile(h2k, fT2, 1, r - 6, tpsB, scrB,
                                       kg1_scalar=True)
                    for i in range(6):
                        transpose_tile(h2k, fT2, 1, 6 + i, tpsB, scrB,
                                       kg1_scalar=True)
                        c_row(i)
                cbp = bctx.enter_context(
                    tc.tile_pool(name="cb_ps", bufs=1, space="PSUM"))
                pcbt = cbp.tile([128, 2, ST], F32, name="pcbt")
                for rT in range(6, ST):
                    c_row(rT)
                d_row(0)
                d_row(1)
                d_row(2)
                fold(cac1, 0)
                for rT in range(3, ST):
                    d_row(rT)
                fold(cac2, 1)

            # ---- epilogue ----
            with tc.tile_pool(name="ep", bufs=1) as ep, \
                 tc.tile_pool(name="ep_ps", bufs=1, space="PSUM") as epp:
                ng = ep.tile([128, NB], F32)
                nc.vector.tensor_reduce(ng[:], acc[:],
                                        axis=mybir.AxisListType.X,
                                        op=ALU.add)
                nc.vector.tensor_add(ng[:, 0:ST], ng[:, 0:ST],
                                     pcbt[:, 0, :])
                nc.vector.tensor_add(ng[:, ST:NB], ng[:, ST:NB],
                                     pcbt[:, 1, :])
                denom = ep.tile([128, NB], F32)
                nc.vector.tensor_scalar_add(denom[:], ng[:], negK0[:])
                lg = ep.tile([128, NB], F32)
                nc.scalar.activation(lg[:], denom[:], AF.Ln)
                if debug_dump:
                    nc.sync.dma_start(ng_dump[:], ng[:])
                    nc.sync.dma_start(poss_dump[:], poss20[:])
                    nc.sync.dma_start(sc8_dump[:], sc8[:])
                ptok = ep.tile([128, NB], F32)
                nc.vector.tensor_mul(ptok[:], lg[:], msk24[:])
                p20m = ep.tile([128, ST], F32)
                nc.vector.tensor_mul(p20m[:], poss20[:], msk[:])
                # poss20 held 64*pos_sim (raw psum); scale to pos_sim/T
                nc.vector.tensor_scalar_mul(p20m[:], p20m[:], EXP_SCALE)
                nc.vector.tensor_sub(ptok[:, 0:ST], ptok[:, 0:ST],
                                     p20m[:])
                nc.vector.tensor_sub(ptok[:, ST:NB], ptok[:, ST:NB],
                                     p20m[:])
                tsum = ep.tile([128, 1], F32)
                nc.vector.tensor_reduce(tsum[:], ptok[:],
                                        axis=mybir.AxisListType.X,
                                        op=ALU.add)
                lps = epp.tile([1, 1], F32)
                nc.tensor.matmul(lps[:], ones_col[:], tsum[:], start=True,
                                 stop=True)
                lsb = ep.tile([1, 1], F32)
                nc.vector.tensor_mul(lsb[:], lps[:], recn[:])
                nc.sync.dma_start(out[:], lsb[:])

    return nc


_NC = None


def _mask_layout(mask_row: np.ndarray) -> np.ndarray:
    # token t = 128 * col + row  ->  [128, ST]
    return np.ascontiguousarray(
        mask_row.astype(np.float32).reshape(ST, 128).T)


def kernel(last_hidden_states_1, last_hidden_states_2, token_mask_batch):
    global _NC
    h1 = np.ascontiguousarray(np.asarray(last_hidden_states_1,
                                         dtype=np.float32))
    h2 = np.ascontiguousarray(np.asarray(last_hidden_states_2,
                                         dtype=np.float32))
    mask = np.asarray(token_mask_batch)
    assert h1.shape == (NCORES, S, D), h1.shape

    if _NC is None:
        _NC = _build(NCORES)

    in_maps = [
        {"h1": h1[b], "h2": h2[b], "maskT": _mask_layout(mask[b])}
        for b in range(NCORES)
    ]
    res = run_bass_kernel_spmd(_NC, in_maps, list(range(NCORES)))
    losses = [float(np.asarray(res.results[b]["loss"]).reshape(()))
              for b in range(NCORES)]
    return np.float32(np.mean(losses))


# revision 36
# speedup vs baseline: 1.4688x; 1.1901x over previous
"""ContraCLM token-level contrastive loss on 8 Trainium2 NeuronCores.

Data-parallel over the batch: core b handles sample b (B=8). Per core,
with S=1536, D=1024, T=0.05, the 2S x 2S exp-sim row sums are built
from three quadrant families, exploiting the symmetry of the full
matrix (only ~70% of the blocks are computed):

  A = f1 f1^T upper triangle: row r covers cols [128r, 1536); row sums
      go to view-1 rows directly, column sums of the strictly-upper
      part are accumulated (DVE, bf16) into cac1 and folded back to
      view-1 rows at the end (they stand in for the mirrored lower
      triangle).
  C = f2 f1^T full rows (computed instead of B so each row block only
      needs one fT2 tile -> no wait on the full view-2 transpose):
      row sums to view-2 rows, column sums into cac1 (these are the
      B-quadrant contributions to view-1 rows).  The diagonal is the
      positive-pair similarity: it is extracted into poss20 via a
      DVE multiply with identity + reduce, and KEPT in the row sum
      (denom = Ng + pos needs exactly that).
  D = f2 f2^T upper triangle, like A, col sums into cac2.

  Self-similarity diagonals get -1e9 added in PSUM before exp -> exact
  zero contribution.  Masked tokens have f=0 (mask folded into the
  rsqrt scale), so each masked column adds exp(0)=1: subtract
  K0 = 2S - 2n.  per_tok = log(rowsum + K0') - pos_sim/T; masked mean;
  each core returns its per-sample mean and the host averages the 8
  scalars (no device collective).

  fp8e4 (x8) DoubleRow matmuls, K=1024 in 4 double-k groups.  exp row
  sums ride the ScalarE activation free-dim accumulator.  View-2 norms
  (sum of squares) run on GpSimd+DVE, interleaved with A' so the
  scalar queue stays clear for exps.
"""

import sys

for _p in ("/opt/trn_rl_repo", "/opt/pypackages"):
    if _p not in sys.path:
        sys.path.append(_p)

from contextlib import ExitStack

import numpy as np

import bass_rust

import concourse.bass as bass
import concourse.tile as tile
from concourse import mybir
from concourse.bass_utils import run_bass_kernel_spmd
from concourse.masks import make_identity
from concourse.vector_clock import ScopedClock

# The walrus build in this container encodes at most 2 sync waits per
# instruction (bass_rust's inst_waits_full agrees), but Tile's semaphore
# assignment can attach more. Hoist excess waits onto unfusable same-engine
# NoOps immediately before the instruction — the engine executes its queue
# in order, so semantics are preserved.
_MAX_WAITS = 1


def _split_excess_waits(nc, ordered):
    for bb_name, insts in ordered.items():
        out = []
        changed = False
        for inst in insts:
            si = getattr(inst, "sync_info", None)
            waits = list(si.on_wait) if si is not None else []
            if len(waits) > _MAX_WAITS:
                changed = True
                extra, keep = waits[:-_MAX_WAITS], waits[-_MAX_WAITS:]
                for i in range(0, len(extra), _MAX_WAITS):
                    out.append(mybir.InstNoOp(
                        name=nc.get_next_instruction_name(),
                        sync_info=mybir.SyncInfo(
                            on_wait=extra[i:i + _MAX_WAITS], on_update=[]),
                        bass_nofuse=True,
                        engine=inst.engine,
                    ))
                si.on_wait = keep
            out.append(inst)
        if changed:
            insts[:] = out


_orig_lower_ordered_insts = tile.TileContext._lower_ordered_insts


def _patched_lower_ordered_insts(self, ordered):
    _split_excess_waits(self.nc, ordered)
    return _orig_lower_ordered_insts(self, ordered)


tile.TileContext._lower_ordered_insts = _patched_lower_ordered_insts


def _split_waits_drain_and_barrier(self, tick_clock, wait_clock):
    nc = self.nc
    probe = nc.sync.nop(nofuse=True)
    wait_clock.add_sem_waits(
        probe.ins, ScopedClock({None: tick_clock.global_clock}))
    si = probe.ins.sync_info
    waits = list(si.on_wait) if si is not None else []
    if len(waits) > _MAX_WAITS:
        si.on_wait = waits[:_MAX_WAITS]
        for i in range(_MAX_WAITS, len(waits), _MAX_WAITS):
            nxt = nc.sync.nop(nofuse=True)
            nxt.ins.sync_info = bass_rust.SyncInfo(
                on_wait=waits[i:i + _MAX_WAITS], on_update=[])
    nc.sync.drain()
    nc.all_engine_barrier()
    assert self.sems is not None
    popped = nc._tile_sem_poison_stack.pop()
    assert popped is self._sem_poison
    nc.clear_and_free_semaphores(list(self.sems.allocated().values()))
    nc.all_engine_barrier()


tile.TileContext._drain_and_barrier = _split_waits_drain_and_barrier

S, D, NCORES = 1536, 1024, 8
ST = S // 128            # 12 s-tiles per view
NB = 2 * ST              # 24 block rows of F
KT = D // 128            # 8 contraction tiles
TEMP_INV = 20.0          # 1 / 0.05
FP8_SCALE = 8.0          # f entries ~N(0, 1/32); x8 keeps them in e4m3's
                         # normal range (|f|*8 <~ 2, well under 240)
EXP_SCALE = TEMP_INV / (FP8_SCALE * FP8_SCALE)
F32 = mybir.dt.float32
BF16 = mybir.dt.bfloat16
FP8 = mybir.dt.float8e4
AF = mybir.ActivationFunctionType
ALU = mybir.AluOpType
DR = mybir.MatmulPerfMode.DoubleRow


def _build(num_devices: int = NCORES, debug_dump: bool = False) -> bass.Bass:
    nc = bass.Bass(num_devices=num_devices)
    h1 = nc.dram_tensor("h1", [S, D], F32, kind="ExternalInput")
    h2 = nc.dram_tensor("h2", [S, D], F32, kind="ExternalInput")
    # mask, pre-laid-out host-side as [128, ST] so token t = 128*col + row
    maskT = nc.dram_tensor("maskT", [128, ST], F32, kind="ExternalInput")
    out = nc.dram_tensor("loss", [1, 1], F32, kind="ExternalOutput")
    if debug_dump:
        ng_dump = nc.dram_tensor("ng_dump", [128, NB], F32,
                                 kind="ExternalOutput")
        poss_dump = nc.dram_tensor("poss_dump", [128, ST], F32,
                                   kind="ExternalOutput")
        sc8_dump = nc.dram_tensor("sc8_dump", [128, NB], F32,
                                  kind="ExternalOutput")

    with tile.TileContext(nc) as tc, ExitStack() as ctx:
        const_pool = ctx.enter_context(tc.tile_pool(name="const", bufs=1))
        big = ctx.enter_context(tc.tile_pool(name="big", bufs=1))
        stat = ctx.enter_context(tc.tile_pool(name="stat", bufs=1))

        h1k = big.tile([128, ST, D], F32)
        h2k = big.tile([128, ST, D], F32)
        fT1 = big.tile([128, KT, S], FP8)        # f1^T * 8, fp8e4
        fT2 = big.tile([128, KT, S], FP8)        # f2^T * 8

        msk = const_pool.tile([128, ST], F32)
        # input DMAs first: they are the long pole at startup
        nc.sync.dma_start(msk[:], maskT[:])
        for t in range(ST):
            nc.sync.dma_start(h1k[:, t, :], h1[t * 128:(t + 1) * 128, :])
        for t in range(ST):
            nc.sync.dma_start(h2k[:, t, :], h2[t * 128:(t + 1) * 128, :])

        identF = const_pool.tile([128, 128], F32)
        make_identity(nc, identF[:])
        identB = const_pool.tile([128, 128], BF16)
        make_identity(nc, identB[:])
        # -1e9 on the diagonal, bf16: injected into self-sim PSUM blocks
        # via an extra accumulating matmul (identB^T @ negIB = -1e9 I)
        negIB = const_pool.tile([128, 128], BF16)
        nc.gpsimd.memset(negIB[:], 0.0)
        nc.gpsimd.affine_select(
            out=negIB[:], in_=negIB[:], compare_op=ALU.not_equal,
            fill=-1e9, base=0, pattern=[[-1, 128]], channel_multiplier=1)
        ones_col = const_pool.tile([128, 1], F32)
        nc.gpsimd.memset(ones_col[:], 1.0)
        ones_sq = const_pool.tile([128, 128], F32)
        nc.gpsimd.memset(ones_sq[:], 1.0)
        ones_bf = const_pool.tile([128, 1], BF16)
        nc.gpsimd.memset(ones_bf[:], 1.0)

        ss = stat.tile([128, NB], F32)           # per-token sum of squares
        sc8 = stat.tile([128, NB], F32)          # 8 * mask * rsqrt(ss)
        nrm = stat.tile([128, NB], F32)
        acc = stat.tile([128, NB, 2], F32)       # per-strip row sums
        cac1 = stat.tile([128, S], BF16)         # col acc -> view-1 rows
        cac2 = stat.tile([128, S], BF16)         # col acc -> view-2 rows
        poss20 = stat.tile([128, ST], F32)       # 64 * pos_sim
        msk24 = stat.tile([128, NB], F32)
        negK0 = stat.tile([128, 1], F32)
        recn = stat.tile([1, 1], F32)

        nc.gpsimd.memset(acc[:], 0.0)
        nc.gpsimd.memset(cac1[:], 0.0)
        nc.gpsimd.memset(cac2[:], 0.0)

        # ---- mask-only precomputes ----
        with tc.tile_pool(name="ep0", bufs=1) as ep0, \
             tc.tile_pool(name="ep0_ps", bufs=1, space="PSUM") as ep0p:
            msum = ep0.tile([128, 1], F32)
            nc.vector.tensor_reduce(msum[:], msk[:],
                                    axis=mybir.AxisListType.X, op=ALU.add)
            nps = ep0p.tile([128, 1], F32)
            nc.tensor.matmul(nps[:], ones_sq[:], msum[:], start=True,
                             stop=True)
            # -K0 = 2n - 2S
            nc.scalar.activation(negK0[:], nps[:], AF.Copy, scale=2.0,
                                 bias=float(-2 * S))
            n2c = ep0.tile([1, 1], F32)
            nc.scalar.activation(n2c[:], nps[0:1, :], AF.Copy, scale=2.0)
            nc.vector.reciprocal(recn[:], n2c[:])   # 1 / (2n)
            nc.vector.tensor_copy(msk24[:, 0:ST], msk[:])
            nc.vector.tensor_copy(msk24[:, ST:NB], msk[:])

        def finish_scale(o, n):
            """sc8[:, o:o+n] = 8 * msk * rsqrt(ss[:, o:o+n])."""
            nc.scalar.activation(nrm[:, o:o + n], ss[:, o:o + n], AF.Sqrt)
            ri = stat.tile([128, n], F32, name=f"ri_{o}")
            nc.vector.reciprocal(ri[:], nrm[:, o:o + n])
            rm = stat.tile([128, n], F32, name=f"rm_{o}")
            nc.vector.tensor_mul(rm[:], ri[:], msk24[:, o:o + n])
            nc.vector.tensor_scalar_mul(sc8[:, o:o + n], rm[:], FP8_SCALE)

        def transpose_tile(hk, fT, half, t, tps, scr, kg1_scalar,
                           kg0_scalar=False):
            """fT[:, :, t*128:+128] = (hk[:,t,:] * sc8)^T as fp8."""
            o = half * ST
            fn = scr.tile([128, D], BF16, tag="fn", name=f"fn_{half}_{t}")
            nc.scalar.activation(fn[:, 0:D // 2], hk[:, t, 0:D // 2],
                                 AF.Copy, scale=sc8[:, o + t:o + t + 1])
            nc.vector.tensor_scalar_mul(fn[:, D // 2:D],
                                        hk[:, t, D // 2:D],
                                        sc8[:, o + t:o + t + 1])
            c0 = t * 128
            for kg in range(2):
                pt = tps.tile([128, 512], BF16, tag="pt",
                              name=f"pt_{half}_{t}_{kg}")
                for j in range(4):
                    k = kg * 4 + j
                    nc.tensor.transpose(pt[:, j * 128:(j + 1) * 128],
                                        fn[:, k * 128:(k + 1) * 128],
                                        identB[:])
                dst = fT[:, kg * 4:(kg + 1) * 4, c0:c0 + 128]
                src = pt[:].rearrange("p (j c) -> p j c", j=4)
                use_scalar = (kg1_scalar if kg == 1 else kg0_scalar)
                if use_scalar:
                    nc.scalar.copy(dst, src)
                else:
                    nc.vector.tensor_copy(dst, src)

        def mm_strip(ps, lhsT, rT, rhsT, col0, ncols):
            """sim strip into ps[:, 0:ncols] (DoubleRow, K=1024)."""
            for g in range(KT // 2):
                u0 = 0
                while u0 < ncols:
                    u1 = min(u0 + 512, ncols)
                    nc.tensor.matmul(
                        ps[:, u0:u1],
                        lhsT[:, 2 * g:2 * g + 2, rT * 128:(rT + 1) * 128],
                        rhsT[:, 2 * g:2 * g + 2, col0 + u0:col0 + u1],
                        perf_mode=DR,
                        start=(g == 0), stop=(g == KT // 2 - 1))
                    u0 = u1

        # ---- phase A: view-1 norms (scalar) + transpose, in halves;
        # view-2 squares (GpSimd) ride along as h2 tiles land ----
        with tc.tile_pool(name="sqpA", bufs=2) as sqp, \
             tc.tile_pool(name="scrA", bufs=3) as scr, \
             tc.tile_pool(name="tpA_ps", bufs=2, space="PSUM") as tps:
            wr = tps.tile([128, 128], BF16, tag="warm", name="warm")

            def pe_keepalive(n):
                # dependency-free transposes: execute only when the
                # tensor queue would otherwise idle, keeping the HAM
                # clock gate at 2.4GHz through the DVE-paced stretches
                for _ in range(n):
                    nc.tensor.transpose(wr[:], identB[:], identB[:])

            pe_keepalive(80)
            for hf in range(2):
                t0 = hf * (ST // 2)
                for t in range(t0, t0 + ST // 2):
                    sq = sqp.tile([128, D], BF16, tag="sq", name=f"sqA_{t}")
                    nc.scalar.activation(sq[:], h1k[:, t, :], AF.Square,
                                         accum_out=ss[:, t:t + 1])
                finish_scale(t0, ST // 2)
                for t in range(t0, t0 + ST // 2):
                    transpose_tile(h1k, fT1, 0, t, tps, scr,
                                   kg1_scalar=False)
                    pe_keepalive(20)

        # view-2 norms (scalar Square+accum) before A' exps hit the
        # scalar queue, so sc8_2 is ready early for the B transposes
        with tc.tile_pool(name="sqpB", bufs=2) as sqpB:
            for t in range(ST):
                sq = sqpB.tile([128, D], BF16, tag="sq", name=f"sqB_{t}")
                nc.scalar.activation(sq[:], h2k[:, t, :], AF.Square,
                                     accum_out=ss[:, ST + t:ST + t + 1])
        finish_scale(ST, ST)

        # ---- A' (A-quadrant upper triangle), phase-B transposes and
        # C rows interleaved to keep TensorE continuously busy ----
        with ExitStack() as bctx:
            psA = bctx.enter_context(
                tc.tile_pool(name="psA", bufs=2, space="PSUM"))
            esA = bctx.enter_context(tc.tile_pool(name="esA", bufs=3))
            scrB = bctx.enter_context(tc.tile_pool(name="scrB", bufs=3))
            dvB = bctx.enter_context(tc.tile_pool(name="dvB", bufs=2))
            if True:

                def a_row(r):
                    ncols = S - r * 128
                    trip = psA.tile([128, S], F32, tag="tp",
                                    name=f"tpA_{r}")
                    mm_strip(trip, fT1, r, fT1, r * 128, ncols)
                    nc.tensor.matmul(trip[:, 0:128], identB[:], negIB[:],
                                     start=False, stop=True,
                                     skip_group_check=True)
                    es = esA.tile([128, S], BF16, tag="es",
                                  name=f"esA_{r}")
                    nc.scalar.activation(es[:, 0:ncols], trip[:, 0:ncols],
                                         AF.Exp, scale=EXP_SCALE,
                                         accum_out=acc[:, r, 0:1])
                    if ncols > 128:
                        nc.gpsimd.tensor_add(cac1[:, (r + 1) * 128:S],
                                             cac1[:, (r + 1) * 128:S],
                                             es[:, 128:ncols])

                def c_row(rT):
                    trip = psA.tile([128, S], F32, tag="tp",
                                    name=f"tpC_{rT}")
                    mm_strip(trip, fT2, rT, fT1, 0, S)
                    # counterpart diagonal: extract 64*pos_sim, keep it
                    # inside the row sum (denom = Ng + pos)
                    dscr = dvB.tile([128, 128], F32, tag="dg",
                                    name=f"dg_{rT}")
                    nc.vector.tensor_mul(
                        dscr[:], trip[:, rT * 128:(rT + 1) * 128],
                        identF[:])
                    nc.vector.tensor_reduce(
                        poss20[:, rT:rT + 1], dscr[:],
                        axis=mybir.AxisListType.X, op=ALU.add)
                    es = esA.tile([128, S], BF16, tag="es",
                                  name=f"esC_{rT}")
                    nc.scalar.activation(es[:], trip[:], AF.Exp,
                                         scale=EXP_SCALE,
                                         accum_out=acc[:, ST + rT, 0:1])
                    nc.vector.tensor_add(cac1[:], cac1[:], es[:])

                def d_row(rT):
                    ncols = S - rT * 128
                    trip = psA.tile([128, S], F32, tag="tp",
                                    name=f"tpD_{rT}")
                    mm_strip(trip, fT2, rT, fT2, rT * 128, ncols)
                    nc.tensor.matmul(trip[:, 0:128], identB[:], negIB[:],
                                     start=False, stop=True,
                                     skip_group_check=True)
                    es = esA.tile([128, S], BF16, tag="es",
                                  name=f"esD_{rT}")
                    nc.scalar.activation(es[:, 0:ncols], trip[:, 0:ncols],
                                         AF.Exp, scale=EXP_SCALE,
                                         accum_out=acc[:, ST + rT, 1:2])
                    if ncols > 128:
                        nc.vector.tensor_add(cac2[:, (rT + 1) * 128:S],
                                             cac2[:, (rT + 1) * 128:S],
                                             es[:, 128:ncols])

                def fold(cac, half):
                    for jb in range(ST):
                        nc.tensor.matmul(
                            pcbt[:, half, jb:jb + 1],
                            cac[:, jb * 128:(jb + 1) * 128],
                            ones_bf[:], start=True, stop=True,
                            skip_group_check=True)

                with tc.tile_pool(name="tpB_ps", bufs=2,
                                  space="PSUM") as tpsB:
                    for r in range(6):
                        a_row(r)
                    for r in range(6, ST):
                        a_row(r)
                        transpose_tile(h2k, fT2, 1, r - 6, tpsB, scrB,
                                       kg1_scalar=False)
                    for i in range(6):
                        transpose_tile(h2k, fT2, 1, 6 + i, tpsB, scrB,
                                       kg1_scalar=False)
                        c_row(i)
                cbp = bctx.enter_context(
                    tc.tile_pool(name="cb_ps", bufs=1, space="PSUM"))
                pcbt = cbp.tile([128, 2, ST], F32, name="pcbt")
                for rT in range(6, ST):
                    c_row(rT)
                d_row(0)
                d_row(1)
                d_row(2)
                fold(cac1, 0)
                for rT in range(3, ST):
                    d_row(rT)
                fold(cac2, 1)

            # ---- epilogue ----
            with tc.tile_pool(name="ep", bufs=1) as ep, \
                 tc.tile_pool(name="ep_ps", bufs=1, space="PSUM") as epp:
                ng = ep.tile([128, NB], F32)
                nc.vector.tensor_reduce(ng[:], acc[:],
                                        axis=mybir.AxisListType.X,
                                        op=ALU.add)
                nc.vector.tensor_add(ng[:, 0:ST], ng[:, 0:ST],
                                     pcbt[:, 0, :])
                nc.vector.tensor_add(ng[:, ST:NB], ng[:, ST:NB],
                                     pcbt[:, 1, :])
                denom = ep.tile([128, NB], F32)
                nc.vector.tensor_scalar_add(denom[:], ng[:], negK0[:])
                lg = ep.tile([128, NB], F32)
                nc.scalar.activation(lg[:], denom[:], AF.Ln)
                if debug_dump:
                    nc.sync.dma_start(ng_dump[:], ng[:])
                    nc.sync.dma_start(poss_dump[:], poss20[:])
                    nc.sync.dma_start(sc8_dump[:], sc8[:])
                ptok = ep.tile([128, NB], F32)
                nc.vector.tensor_mul(ptok[:], lg[:], msk24[:])
                p20m = ep.tile([128, ST], F32)
                nc.vector.tensor_mul(p20m[:], poss20[:], msk[:])
                # poss20 held 64*pos_sim (raw psum); scale to pos_sim/T
                nc.vector.tensor_scalar_mul(p20m[:], p20m[:], EXP_SCALE)
                nc.vector.tensor_sub(ptok[:, 0:ST], ptok[:, 0:ST],
                                     p20m[:])
                nc.vector.tensor_sub(ptok[:, ST:NB], ptok[:, ST:NB],
                                     p20m[:])
                tsum = ep.tile([128, 1], F32)
                nc.vector.tensor_reduce(tsum[:], ptok[:],
                                        axis=mybir.AxisListType.X,
                                        op=ALU.add)
                lps = epp.tile([1, 1], F32)
                nc.tensor.matmul(lps[:], ones_col[:], tsum[:], start=True,
                                 stop=True)
                lsb = ep.tile([1, 1], F32)
                nc.vector.tensor_mul(lsb[:], lps[:], recn[:])
                nc.sync.dma_start(out[:], lsb[:])

    return nc


_NC = None


def _mask_layout(mask_row: np.ndarray) -> np.ndarray:
    # token t = 128 * col + row  ->  [128, ST]
    return np.ascontiguousarray(
        mask_row.astype(np.float32).reshape(ST, 128).T)


def kernel(last_hidden_states_1, last_hidden_states_2, token_mask_batch):
    global _NC
    h1 = np.ascontiguousarray(np.asarray(last_hidden_states_1,
                                         dtype=np.float32))
    h2 = np.ascontiguousarray(np.asarray(last_hidden_states_2,
                                         dtype=np.float32))
    mask = np.asarray(token_mask_batch)
    assert h1.shape == (NCORES, S, D), h1.shape

    if _NC is None:
        _NC = _build(NCORES)

    in_maps = [
        {"h1": h1[b], "h2": h2[b], "maskT": _mask_layout(mask[b])}
        for b in range(NCORES)
    ]
    res = run_bass_kernel_spmd(_NC, in_maps, list(range(NCORES)))
    losses = [float(np.asarray(res.results[b]["loss"]).reshape(()))
              for b in range(NCORES)]
    return np.float32(np.mean(losses))
